# revision 1
# baseline (speedup 1.0000x reference)
"""BinaryConnect 3x3 SAME conv (NHWC, 32x112x112x128 -> 32x112x112x256) on 8 trn2 cores.

Strategy (data-parallel, 4 images per core):
  - Host: binarize kernel to +/-1 fp16 (exact), cast x to fp16, transpose to
    channel-major [cin, n, hp, wp] with 1-px zero halo, rows padded to 114 and
    one zero tail row (115 rows total), flattened per image.
  - Device: out[cout_half, fo] accumulates 9 matmuls (one per 3x3 tap) in PSUM:
    lhsT = wb_tap [cin=128, cout_half=128] (stationary),
    rhs  = x_flat[cin=128, fo + dh*114 + dw : +S] (moving, S=456),
    fp16 in, fp32 PSUM accumulate. Output written channel-major, un-transposed
    on host. Padded output columns (w=112,113) are garbage and stripped on host.
"""

import os

import numpy as np

import concourse.bass as bass
import concourse.mybir as mybir
import concourse.tile as tile
from concourse import bacc
from concourse.bass_utils import run_bass_kernel_spmd

N_CORES = 8
NPC = 4            # images per core
H = 112
WP = 114           # padded row width
HP = 115           # 1 top pad + 112 rows + 1 bottom pad + 1 zero tail row
XF = HP * WP       # 13110 flat padded-input positions per image
FO = H * WP        # 12768 flat padded-output positions per image
S = 456            # matmul free dim (4*114, divides FO; <=512 fp32 PSUM bank)
TS = FO // S       # 28 spatial tiles per image
CI = 128
CO = 256

_nc_cache = None
LAST_RESULT = None


def _build():
    nc = bacc.Bacc(
        "TRN2",
        target_bir_lowering=False,
        debug=False,
        num_devices=N_CORES,
    )
    x_d = nc.dram_tensor("xp", [CI, NPC, XF], mybir.dt.float16, kind="ExternalInput")
    w_d = nc.dram_tensor("wt", [CI, 9 * CO], mybir.dt.float16, kind="ExternalInput")
    o_d = nc.dram_tensor(
        "out_cm", [CO, NPC, FO], mybir.dt.float32, kind="ExternalOutput"
    )
    offs = [dh * WP + dw for dh in range(3) for dw in range(3)]
    with tile.TileContext(nc) as tc:
        with (
            tc.tile_pool(name="xpool", bufs=1) as xpool,
            tc.tile_pool(name="wpool", bufs=1) as wpool,
            tc.tile_pool(name="psum", bufs=8, space=bass.MemorySpace.PSUM) as psum,
            tc.tile_pool(name="opool", bufs=8) as opool,
        ):
            wt_s = wpool.tile([CI, 9 * CO], mybir.dt.float16)
            nc.sync.dma_start(wt_s[:], w_d[:, :])
            xs = []
            for n in range(NPC):
                xt = xpool.tile([CI, XF], mybir.dt.float16, tag=f"x{n}", name=f"x{n}")
                nc.sync.dma_start(xt[:], x_d[:, n, :])
                xs.append(xt)
            for n in range(NPC):
                for st in range(TS):
                    s0 = st * S
                    for half in range(2):
                        ps = psum.tile([128, S], mybir.dt.float32, name="ps")
                        for t in range(9):
                            w0 = t * CO + half * 128
                            nc.tensor.matmul(
                                ps[:],
                                wt_s[:, w0 : w0 + 128],
                                xs[n][:, s0 + offs[t] : s0 + offs[t] + S],
                                start=(t == 0),
                                stop=(t == 8),
                            )
                        ot = opool.tile([128, S], mybir.dt.float32, name="ot")
                        nc.vector.tensor_copy(ot[:], ps[:])
                        nc.sync.dma_start(
                            o_d[half * 128 : half * 128 + 128, n, s0 : s0 + S], ot[:]
                        )
    nc.compile()
    return nc


def _get_nc():
    global _nc_cache
    if _nc_cache is None:
        _nc_cache = _build()
    return _nc_cache


def kernel(x, kernel):
    global LAST_RESULT
    x = np.asarray(x)
    k = np.asarray(kernel)

    wb = np.where(k >= 0, np.float16(1), np.float16(-1))  # [3,3,128,256]
    wt = np.ascontiguousarray(wb.transpose(2, 0, 1, 3).reshape(CI, 9 * CO))

    x16 = x.astype(np.float16)  # [32,112,112,128]
    in_maps = []
    for c in range(N_CORES):
        xp = np.zeros((CI, NPC, HP, WP), np.float16)
        xp[:, :, 1:113, 1:113] = x16[c * NPC : (c + 1) * NPC].transpose(3, 0, 1, 2)
        in_maps.append({"xp": xp.reshape(CI, NPC, XF), "wt": wt})

    nc = _get_nc()
    trace = os.environ.get("BCONV_TRACE", "0") == "1"
    res = run_bass_kernel_spmd(
        nc, in_maps, core_ids=list(range(N_CORES)), trace=trace
    )
    LAST_RESULT = res

    out = np.empty((32, H, H, CO), np.float32)
    for c in range(N_CORES):
        o = res.results[c]["out_cm"]  # [256, 4, 12768]
        o = o.reshape(CO, NPC, H, WP)[:, :, :, :112]
        out[c * NPC : (c + 1) * NPC] = o.transpose(1, 2, 3, 0)
    return out
